# revision 19
# baseline (speedup 1.0000x reference)
"""LinkPredictor (GNN edge scorer) Bass kernel for 8 Trainium2 NeuronCores.

score[e] = W2 @ relu(W1 @ [h[src[e]]; h[dst[e]]] + b1) + b2

Strategy (pure data parallel over edges, per sharding hint):
  - shard E=1.6M edges across 8 cores (200k each)
  - replicate h (bf16) and MLP weights
  - gather h rows with the gpsimd dma_gather ucode (256B rows, transposed
    into [feat, edge] layout on the fly). dma_gather indices are int16, so
    h is addressed through 4 static 32768-row windows and the host buckets
    each core's edges by (src>>15, dst>>15) into 16 buckets padded to
    2048-edge groups (statically capped group counts; numpy fallback for
    the ~10-sigma overflow case).
  - per 512-edge subtile: bf16 matmuls for both layers, ScalarE fused
    bias+relu, DVE +b2 into an SBUF staging row, one DMA out per group.
  - host un-permutes the bucketed scores back to edge order.
"""

import numpy as np

N_NODES = 100000
N_EDGES = 1600000
D = 128
H = 256
N_CORES = 8
E_PER_CORE = N_EDGES // N_CORES  # 200000

CHUNK_BITS = 15
CHUNK = 1 << CHUNK_BITS          # 32768 rows per gather window
N_CHUNKS = 4                     # ceil(100000 / 32768)
SUB = 512                        # edges per matmul subtile

# slots per (src_chunk, dst_chunk) bucket; chunks 0-2 are full 32768-row
# windows (p=.32768 each), chunk 3 has 1696 rows (p=.01696).
# big-big buckets (mu=21477, sigma=138): 11 groups of 2048 (+7.6 sigma).
# buckets touching chunk 3 (mu<=1111, sigma<=33): 2048-edge groups waste
# ~45% padding, so they use 512-edge gather groups instead (+12.9 sigma).
_CAPS_E = np.empty((4, 4), dtype=np.int64)
_GSIZE = np.empty((4, 4), dtype=np.int64)
for _i in range(4):
    for _j in range(4):
        if _i < 3 and _j < 3:
            _CAPS_E[_i, _j] = 11 * 2048
            _GSIZE[_i, _j] = 2048
        else:
            _CAPS_E[_i, _j] = 1536 if (_i < 3 or _j < 3) else 512
            _GSIZE[_i, _j] = 512
CAPS_E = _CAPS_E.reshape(-1)                   # [16] slots per bucket
E_PAD = int(CAPS_E.sum())                      # 212480 slots per core
EOFF = np.concatenate([[0], np.cumsum(CAPS_E)[:-1]])  # slot offset per bucket
# static group table: (slot offset, size, src_chunk, dst_chunk)
GROUPS = []
for _b in range(16):
    _gs = int(_GSIZE.reshape(-1)[_b])
    for _k in range(int(CAPS_E[_b]) // _gs):
        GROUPS.append((int(EOFF[_b]) + _k * _gs, _gs, _b // 4, _b % 4))
NG = len(GROUPS)                               # 118 groups per core

_cache = {}


def _build_nc():
    from contextlib import ExitStack

    import concourse.bass as bass  # noqa: F401
    import concourse.tile as tile
    from concourse import bacc, mybir

    f32 = mybir.dt.float32
    bf16 = mybir.dt.bfloat16
    i16 = mybir.dt.int16

    nc = bacc.Bacc("TRN2", target_bir_lowering=False, debug=False)

    h_d = nc.dram_tensor("h", [N_NODES, D], bf16, kind="ExternalInput")
    isrc_d = nc.dram_tensor("isrc", [128, E_PAD // 16], i16, kind="ExternalInput")
    idst_d = nc.dram_tensor("idst", [128, E_PAD // 16], i16, kind="ExternalInput")
    w1s_d = nc.dram_tensor("w1s", [D, H], bf16, kind="ExternalInput")
    w1d_d = nc.dram_tensor("w1d", [D, H], bf16, kind="ExternalInput")
    b1_d = nc.dram_tensor("b1", [D, 2], f32, kind="ExternalInput")
    w2_d = nc.dram_tensor("w2", [D, 2], bf16, kind="ExternalInput")
    b2_d = nc.dram_tensor("b2", [1, 1], f32, kind="ExternalInput")
    out_d = nc.dram_tensor("out", [1, E_PAD], f32, kind="ExternalOutput")

    relu = mybir.ActivationFunctionType.Relu

    # static gather windows into h
    row0 = [0, CHUNK, 2 * CHUNK, 3 * CHUNK]
    row1 = [CHUNK, 2 * CHUNK, 3 * CHUNK, N_NODES]

    with tile.TileContext(nc) as tc, ExitStack() as ctx:
        const = ctx.enter_context(tc.tile_pool(name="const", bufs=1))
        idxp = ctx.enter_context(tc.tile_pool(name="idx", bufs=3))
        gp = ctx.enter_context(tc.tile_pool(name="gather", bufs=3))
        rp = ctx.enter_context(tc.tile_pool(name="relu", bufs=2))
        scp = ctx.enter_context(tc.tile_pool(name="score", bufs=3))
        mm_ps = ctx.enter_context(tc.tile_pool(name="mm_ps", bufs=2, space="PSUM"))
        sc_ps = ctx.enter_context(tc.tile_pool(name="sc_ps", bufs=2, space="PSUM"))

        w1s_t = const.tile([D, H], bf16)
        w1d_t = const.tile([D, H], bf16)
        b1_t = const.tile([D, 2], f32)
        w2_t = const.tile([D, 2], bf16)
        b2_t = const.tile([1, 1], f32)
        nc.sync.dma_start(w1s_t[:], w1s_d[:])
        nc.sync.dma_start(w1d_t[:], w1d_d[:])
        nc.sync.dma_start(b1_t[:], b1_d[:])
        nc.sync.dma_start(w2_t[:], w2_d[:])
        nc.sync.dma_start(b2_t[:], b2_d[:])

        for g, (off, gsz, cs, cd) in enumerate(GROUPS):
            swin = h_d[row0[cs]:row1[cs], :]
            dwin = h_d[row0[cd]:row1[cd], :]
            csl = slice(off // 16, (off + gsz) // 16)

            is_t = idxp.tile([128, gsz // 16], i16, tag=f"is{gsz}")
            id_t = idxp.tile([128, gsz // 16], i16, tag=f"id{gsz}")
            nc.sync.dma_start(is_t[:], isrc_d[:, csl])
            nc.sync.dma_start(id_t[:], idst_d[:, csl])

            gs = gp.tile([128, 1, gsz], bf16, tag=f"gs{gsz}")
            gd = gp.tile([128, 1, gsz], bf16, tag=f"gd{gsz}")
            nc.gpsimd.dma_gather(
                gs[:], swin, is_t[:], gsz, gsz, D, transpose=True,
                single_packet=False)
            nc.gpsimd.dma_gather(
                gd[:], dwin, id_t[:], gsz, gsz, D, transpose=True,
                single_packet=False)

            sco = scp.tile([1, gsz], f32, tag=f"sco{gsz}")
            for j in range(gsz // SUB):
                sl = slice(j * SUB, (j + 1) * SUB)
                r0 = mm_ps.tile([128, SUB], f32, tag="r0")
                r1 = mm_ps.tile([128, SUB], f32, tag="r1")
                nc.tensor.matmul(r0[:], lhsT=w1s_t[:, 0:128], rhs=gs[:, 0, sl],
                                 start=True, stop=False)
                nc.tensor.matmul(r0[:], lhsT=w1d_t[:, 0:128], rhs=gd[:, 0, sl],
                                 start=False, stop=True)
                nc.tensor.matmul(r1[:], lhsT=w1s_t[:, 128:256], rhs=gs[:, 0, sl],
                                 start=True, stop=False)
                nc.tensor.matmul(r1[:], lhsT=w1d_t[:, 128:256], rhs=gd[:, 0, sl],
                                 start=False, stop=True)

                R0 = rp.tile([128, SUB], bf16, tag="R0")
                R1 = rp.tile([128, SUB], bf16, tag="R1")
                nc.scalar.activation(R0[:], r0[:], relu, bias=b1_t[:, 0:1], scale=1.0)
                nc.scalar.activation(R1[:], r1[:], relu, bias=b1_t[:, 1:2], scale=1.0)

                sc = sc_ps.tile([1, SUB], f32, tag="sc")
                nc.tensor.matmul(sc[:], lhsT=w2_t[:, 0:1], rhs=R0[:],
                                 start=True, stop=False)
                nc.tensor.matmul(sc[:], lhsT=w2_t[:, 1:2], rhs=R1[:],
                                 start=False, stop=True)

                nc.vector.tensor_scalar(out=sco[:, sl], in0=sc[:], scalar1=b2_t[:],
                                        scalar2=None, op0=mybir.AluOpType.add)

            nc.sync.dma_start(out_d[0:1, off:off + gsz], sco[:])

    nc.compile()
    return nc


def _get_nc():
    if "nc" not in _cache:
        _cache["nc"] = _build_nc()
    return _cache["nc"]


def _prep_core(s, d):
    """Bucket one core's edges; returns packed int16 idx tensors for the
    device, the slot of each kept edge, and the overflow edge list."""
    e = s.shape[0]
    cs = s >> CHUNK_BITS
    cd = d >> CHUNK_BITS
    b = cs * 4 + cd
    counts = np.bincount(b, minlength=16)
    cum = np.concatenate([[0], np.cumsum(counts)[:-1]])
    # sort by (bucket, src) so src-gather addresses ascend within each call
    # (HBM page locality); dst stays random.
    order = np.lexsort((s, b))
    pos = np.empty(e, dtype=np.int64)
    pos[order] = np.arange(e) - cum[b[order]]
    kept = pos < CAPS_E[b]
    slot = EOFF[b] + pos  # valid where kept

    arr_s = np.zeros(E_PAD, dtype=np.int16)
    arr_d = np.zeros(E_PAD, dtype=np.int16)
    ks = slot[kept]
    arr_s[ks] = (s[kept] & (CHUNK - 1)).astype(np.int16)
    arr_d[ks] = (d[kept] & (CHUNK - 1)).astype(np.int16)

    def pack(a):
        # slot off+i of group (off, gsz) -> column off//16 + i//16 of
        # partition i % 16, replicated x8 over the 128 partitions
        cols = np.empty((16, E_PAD // 16), dtype=np.int16)
        for off, gsz, _, _ in GROUPS:
            blk = a[off:off + gsz].reshape(gsz // 16, 16).T  # [16, gsz//16]
            cols[:, off // 16:(off + gsz) // 16] = blk
        return np.ascontiguousarray(np.tile(cols, (8, 1)))  # [128, E_PAD//16]

    return pack(arr_s), pack(arr_d), slot, kept


def _make_runner(nc):
    """Replicates bass2jax.run_bass_via_pjrt's multi-core shard_map path but
    returns a reusable jitted callable so repeated (timed) runs are possible."""
    import jax
    import numpy as _np
    from jax.sharding import Mesh, PartitionSpec
    from jax.experimental.shard_map import shard_map

    import concourse.mybir as mybir
    from concourse.bass2jax import (
        _bass_exec_p, install_neuronx_cc_hook, partition_id_tensor)

    install_neuronx_cc_hook()

    partition_name = (
        nc.partition_id_tensor.name if nc.partition_id_tensor else None)
    in_names, out_names, out_avals, zero_outs = [], [], [], []
    for alloc in nc.m.functions[0].allocations:
        if not isinstance(alloc, mybir.MemoryLocationSet):
            continue
        name = alloc.memorylocations[0].name
        if alloc.kind == "ExternalInput":
            if name != partition_name:
                in_names.append(name)
        elif alloc.kind == "ExternalOutput":
            out_names.append(name)
            shape = tuple(alloc.tensor_shape)
            dtype = mybir.dt.np(alloc.dtype)
            out_avals.append(jax.core.ShapedArray(shape, dtype))
            zero_outs.append(_np.zeros(shape, dtype))
    n_params = len(in_names)
    n_outs = len(out_avals)
    all_names = in_names + out_names
    if partition_name is not None:
        all_names = all_names + [partition_name]
    donate = tuple(range(n_params, n_params + n_outs))

    def _body(*args):
        operands = list(args)
        if partition_name is not None:
            operands.append(partition_id_tensor())
        outs = _bass_exec_p.bind(
            *operands,
            out_avals=tuple(out_avals),
            in_names=tuple(all_names),
            out_names=tuple(out_names),
            lowering_input_output_aliases=(),
            sim_require_finite=True,
            sim_require_nnan=True,
            nc=nc,
        )
        return tuple(outs)

    devices = jax.devices()[:N_CORES]
    mesh = Mesh(np.asarray(devices), ("core",))
    sharded = jax.jit(
        shard_map(_body, mesh=mesh,
                  in_specs=(PartitionSpec("core"),) * (n_params + n_outs),
                  out_specs=(PartitionSpec("core"),) * n_outs,
                  check_rep=False),
        donate_argnums=donate, keep_unused=True)
    return sharded, in_names, out_names, out_avals, zero_outs


def kernel(h, src, dst, W1_w, W1_b, W2_w, W2_b, _time_iters=0):
    import jax
    import ml_dtypes

    nc = _get_nc()

    h32 = np.ascontiguousarray(np.asarray(h, dtype=np.float32))
    hbf = h32.astype(ml_dtypes.bfloat16)
    w1 = np.asarray(W1_w, dtype=np.float32)          # [H, 2D]
    w1s = np.ascontiguousarray(w1[:, 0:D].T).astype(ml_dtypes.bfloat16)
    w1dm = np.ascontiguousarray(w1[:, D:2 * D].T).astype(ml_dtypes.bfloat16)
    b1 = np.asarray(W1_b, dtype=np.float32)
    b1p = np.ascontiguousarray(b1.reshape(2, D).T)   # [128, 2]
    w2 = np.asarray(W2_w, dtype=np.float32).reshape(H)
    w2p = np.ascontiguousarray(w2.reshape(2, D).T).astype(ml_dtypes.bfloat16)
    b2 = np.asarray(W2_b, dtype=np.float32).reshape(1, 1)

    src_l = np.asarray(src, dtype=np.int64)
    dst_l = np.asarray(dst, dtype=np.int64)

    in_maps, prep = [], []
    for c in range(N_CORES):
        sl = slice(c * E_PER_CORE, (c + 1) * E_PER_CORE)
        ps, pd, slot, kept = _prep_core(src_l[sl], dst_l[sl])
        prep.append((slot, kept))
        in_maps.append({
            "h": hbf, "isrc": ps, "idst": pd,
            "w1s": w1s, "w1d": w1dm, "b1": b1p, "w2": w2p, "b2": b2,
        })

    if "runner" not in _cache:
        _cache["runner"] = _make_runner(nc)
    sharded, in_names, out_names, out_avals, zero_outs = _cache["runner"]

    concat_in = [
        np.concatenate([in_maps[c][name] for c in range(N_CORES)], axis=0)
        for name in in_names
    ]
    concat_zeros = [
        np.zeros((N_CORES * z.shape[0], *z.shape[1:]), z.dtype) for z in zero_outs
    ]
    out_arrs = sharded(*concat_in, *concat_zeros)
    jax.block_until_ready(out_arrs)

    if _time_iters > 0:
        import time
        from jax.sharding import Mesh, NamedSharding, PartitionSpec
        mesh = Mesh(np.asarray(jax.devices()[:N_CORES]), ("core",))
        sh = NamedSharding(mesh, PartitionSpec("core"))
        dev_in = [jax.device_put(a, sh) for a in concat_in]
        jax.block_until_ready(dev_in)

        def _fresh_zeros():
            return [jax.device_put(
                np.zeros((N_CORES * z.shape[0], *z.shape[1:]), z.dtype), sh)
                for z in zero_outs]

        # single-call (blocking) wall times: include ~60-70ms of axon-tunnel
        # round-trip latency on top of device execution
        times = []
        for _ in range(_time_iters):
            zd = _fresh_zeros()
            jax.block_until_ready(zd)
            t0 = time.perf_counter()
            o = sharded(*dev_in, *zd)
            jax.block_until_ready(o)
            times.append(time.perf_counter() - t0)
        kernel.exec_times_s = times

        # pipelined differencing isolates per-execution device time from the
        # fixed tunnel latency: launch K back-to-back (device serializes NEFF
        # executions), block once; marginal = (t_K - t_1) / (K - 1)
        K = 9
        best = None
        for _ in range(2):
            z1 = _fresh_zeros()
            jax.block_until_ready(z1)
            t0 = time.perf_counter()
            o = sharded(*dev_in, *z1)
            jax.block_until_ready(o)
            t1 = time.perf_counter() - t0
            zks = [_fresh_zeros() for _ in range(K)]
            for zk in zks:
                jax.block_until_ready(zk)
            t0 = time.perf_counter()
            os = [sharded(*dev_in, *zk) for zk in zks]
            jax.block_until_ready(os)
            tk = time.perf_counter() - t0
            marg = (tk - t1) / (K - 1)
            if marg > 0 and (best is None or marg < best):
                best = marg
        if best is not None:
            kernel.hw_time_s = best

    oi = out_names.index("out")
    full = np.asarray(out_arrs[oi]).reshape(N_CORES, *out_avals[oi].shape)
    out = np.empty(N_EDGES, dtype=np.float32)
    for c in range(N_CORES):
        slot, kept = prep[c]
        raw = full[c].reshape(-1)
        seg = out[c * E_PER_CORE:(c + 1) * E_PER_CORE]
        seg[kept] = raw[slot[kept]]
        if not kept.all():  # statistically-impossible bucket overflow
            idx = np.nonzero(~kept)[0]
            s_n = src_l[c * E_PER_CORE + idx]
            d_n = dst_l[c * E_PER_CORE + idx]
            x = np.concatenate([h32[s_n], h32[d_n]], axis=1)
            hid = np.maximum(x @ np.asarray(W1_w, np.float32).T + b1, 0.0)
            seg[idx] = hid @ w2 + b2[0, 0]
    return out
